# revision 25
# baseline (speedup 1.0000x reference)
"""Bahdanau additive attention kernel for Trainium2 (8 NeuronCores).

Problem shapes (hardcoded): B=4, Q=256, V=2048, H=512, U=128, fp32.

reference:
    pq = queries @ w1                  # [B,Q,U]
    pv = values  @ w2                  # [B,V,U]
    scores[b,q,v] = sum_u tanh(pq[b,q,u] + pv[b,v,u]) * v[u]
    attn = softmax(scores, axis=-1)
    out  = attn @ values               # [B,Q,H]

Sharding: 8 cores = 4 batches x 2 query-halves. Each core handles a full
softmax over V for its [128, H] query slice -> no collectives needed.

Per-core dataflow (ACT-roofline design: the 128*2048*128 tanh evals run
as 128 dense back-to-back ACTIVATE instructions, everything else hides
underneath):
  - pqT [U, Qloc] and pvT [U, V] via PE projections; host supplies
    transposed queries/values so no on-chip transposes are needed on the
    critical path. pvT accumulates directly in PSUM (faster ACT source).
  - loop over groups g of 4 q's (one per 32-col PE strip):
      ACT: t_q = tanh(pvT + pqT[:,q] bias)  -> [128,2048] fp16 in SBUF
      PE : 16 col-tiled matvecs (tile_position=(0,32s)) accumulate
           scores rows into PSUM; all four strips run concurrently and
           share one M=32 stationary window (v at window-col g).
  - softmax: exp (no max-subtract needed; |scores| <= sum|v| ~ 9) with
    accum_out giving the row sums for free; DVE reciprocal.
  - out = (eT @ values) * 1/sum: 16 PE transposes of e (fp16) + 16
    accumulating fp16 matmuls against host-supplied fp16 values tiles.
"""

from contextlib import ExitStack

import numpy as np

import concourse.bacc as bacc
import concourse.bass as bass
import concourse.tile as tile
from concourse import mybir
from concourse.masks import make_identity

B, Q, V, H, U = 4, 256, 2048, 512, 128
QL = Q // 2            # per-core queries
VT = V // 128          # 16 value tiles
HT = H // 128          # 4 hidden tiles
NB = V // 512          # 4 psum bank chunks of the scores row

F32 = mybir.dt.float32
F16 = mybir.dt.float16


def build_nc(t_dtype=F16):
    nc = bacc.Bacc("TRN2", target_bir_lowering=False, debug=False)
    F32R = mybir.dt.float32r
    qT_ext = nc.declare_dram_parameter("qT", [HT, 128, QL], F32, isOutput=False)
    valsT_ext = nc.declare_dram_parameter(
        "valsT", [NB, HT, 128, 512], F32R, isOutput=False)
    vals16_ext = nc.declare_dram_parameter("vals16", [VT, 128, H], F16, isOutput=False)
    w1_ext = nc.declare_dram_parameter("w1", [HT, 128, U], F32, isOutput=False)
    w2_ext = nc.declare_dram_parameter("w2", [HT, 128, U], F32R, isOutput=False)
    v_ext = nc.declare_dram_parameter("v", [U, 1], F32, isOutput=False)
    out_ext = nc.declare_dram_parameter("out", [QL, H], F32, isOutput=True)

    with tile.TileContext(nc) as tc, ExitStack() as ctx:
        singles = ctx.enter_context(tc.tile_pool(name="singles", bufs=1))
        work = ctx.enter_context(tc.tile_pool(name="work", bufs=3))
        apool = ctx.enter_context(tc.tile_pool(name="adds", bufs=4))
        tpool = ctx.enter_context(tc.tile_pool(name="tanh", bufs=2))

        # --- inputs; one dma_start per tensor (a single DMA already
        # fans out over all 16 SDMA engines). valsT arrives in 4 v-chunks
        # so the pv build can pipeline behind the transfers. ------------
        sb_w2 = singles.tile([128, HT, U], F32R)
        nc.sync.dma_start(out=sb_w2, in_=w2_ext.rearrange("t p u -> p t u"))
        sb_w1 = singles.tile([128, HT, U], F32)
        nc.sync.dma_start(out=sb_w1, in_=w1_ext.rearrange("t p u -> p t u"))
        sb_qT = singles.tile([128, HT, QL], F32)
        nc.sync.dma_start(out=sb_qT, in_=qT_ext.rearrange("t p q -> p t q"))
        sb_v = singles.tile([128, 1], F32)
        nc.sync.dma_start(out=sb_v, in_=v_ext[:])
        sb_valsT = singles.tile([128, NB, HT, 512], F32R)
        for c in range(NB):
            nc.sync.dma_start(
                out=sb_valsT[:, c, :, :],
                in_=valsT_ext[c].rearrange("t p j -> p t j"))
        sb_vals16 = singles.tile([128, VT, H], F16)
        nc.sync.dma_start(out=sb_vals16, in_=vals16_ext.rearrange("t p h -> p t h"))

        # v embedded at column 32 of a zero pad; the M=32 window
        # [:, 32-g:64-g] puts v at window-column g, so the matvec result
        # lands in row g of a 32-partition PSUM strip.
        sb_vpad = singles.tile([128, 64], t_dtype)
        nc.vector.memset(sb_vpad, 0.0)
        nc.vector.tensor_copy(out=sb_vpad[:, 32:33], in_=sb_v)
        identity16 = singles.tile([128, 128], F16)
        make_identity(nc, identity16)

        # --- pqT [u, q] -----------------------------------------------
        sb_pqT = singles.tile([128, QL], F32)
        with tc.tile_pool(name="ps_pq", bufs=1, space="PSUM") as pqpool:
            ps_pq = pqpool.tile([128, QL], F32)
            for ht in range(HT):
                nc.tensor.matmul(
                    ps_pq, lhsT=sb_w1[:, ht, :], rhs=sb_qT[:, ht, :],
                    start=(ht == 0), stop=(ht == HT - 1),
                )
            nc.vector.tensor_copy(out=sb_pqT, in_=ps_pq)

        with tc.tile_pool(name="ps_scores", bufs=1, space="PSUM") as scpool:
            psum_scores = scpool.tile([128, V], F32)

            # --- pvT [u, v] built via PSUM, copied to SBUF ------------
            sb_pvT = singles.tile([128, V], F32)
            with tc.tile_pool(name="ps_pvt", bufs=2, space="PSUM") as pvpool:
                for c in range(NB):
                    ps_pv = pvpool.tile([128, 512], F32, tag="pv")
                    for ht in range(HT):
                        nc.tensor.matmul(
                            ps_pv,
                            lhsT=sb_w2[:, ht, :],
                            rhs=sb_valsT[:, c, ht, :],
                            start=(ht == 0), stop=(ht == HT - 1),
                        )
                    nc.vector.tensor_copy(
                        out=sb_pvT[:, c * 512:(c + 1) * 512], in_=ps_pv)

            # --- main loop -------------------------------------------
            # Per group of 4 q's (one per 32-col PE strip):
            #   DVE: 4 per-partition adds  pvT + pqT[:,q]  (2x mode)
            #   ACT: one fused tanh over [128, 4*2048] (amortizes the
            #        per-instruction overhead 4x; ACT is the roofline)
            #   PE : 16 col-tiled matvecs accumulate score rows
            for g in range(32):
                addbuf = apool.tile([128, 4, V], F16, tag="add")
                t_t = tpool.tile([128, 4, V], t_dtype, tag="t")
                if g == 0:
                    # Pipeline the first group per 512-col chunk so the
                    # ACT stream starts as soon as pvT chunk 0 lands.
                    for c in range(NB):
                        cs = slice(c * 512, (c + 1) * 512)
                        for s in range(4):
                            q = 32 * s + g
                            nc.vector.tensor_scalar_add(
                                addbuf[:, s, cs], sb_pvT[:, cs],
                                sb_pqT[:, q:q + 1])
                        nc.scalar.activation(
                            out=t_t[:, :, cs], in_=addbuf[:, :, cs],
                            func=mybir.ActivationFunctionType.Tanh,
                        )
                else:
                    for s in range(4):
                        q = 32 * s + g
                        nc.vector.tensor_scalar_add(
                            addbuf[:, s, :], sb_pvT, sb_pqT[:, q:q + 1])
                    nc.scalar.activation(
                        out=t_t, in_=addbuf,
                        func=mybir.ActivationFunctionType.Tanh,
                    )
                for nb in range(NB):
                    for s in range(4):
                        nc.tensor.matmul(
                            psum_scores[32 * s:32 * s + 32,
                                        nb * 512:(nb + 1) * 512],
                            lhsT=sb_vpad[:, 32 - g:64 - g],
                            rhs=t_t[:, s, nb * 512:(nb + 1) * 512],
                            start=(g == 0), stop=(g == 31),
                            tile_position=(0, 32 * s),
                            skip_group_check=True,
                        )

            # --- softmax (no max-subtract; scores bounded by sum|v|) --
            # Split in halves so the eT transposes can start early.
            sb_e = singles.tile([128, V], F16)
            sb_sum0 = work.tile([128, 1], F32)
            sb_sum1 = work.tile([128, 1], F32)
            nc.scalar.activation(
                out=sb_e[:, :V // 2], in_=psum_scores[:, :V // 2],
                func=mybir.ActivationFunctionType.Exp,
                bias=0.0, scale=1.0, accum_out=sb_sum0,
            )
            nc.scalar.activation(
                out=sb_e[:, V // 2:], in_=psum_scores[:, V // 2:],
                func=mybir.ActivationFunctionType.Exp,
                bias=0.0, scale=1.0, accum_out=sb_sum1,
            )
            sb_sum = work.tile([128, 1], F32)
            nc.vector.tensor_add(sb_sum, sb_sum0, sb_sum1)
            sb_rsum = work.tile([128, 1], F32)
            nc.vector.reciprocal(sb_rsum, sb_sum)

        # --- out = diag(1/sum) @ e @ values ---------------------------
        with tc.tile_pool(name="ps_tail", bufs=4, space="PSUM") as tailpool:
            ps_out = tailpool.tile([128, H], F32, tag="ps_out")
            for vt in range(VT):
                ps_tr = tailpool.tile([128, 128], F16, tag="ps_tr")
                nc.tensor.transpose(
                    ps_tr, sb_e[:, vt * 128:(vt + 1) * 128], identity16)
                sb_eT_t = work.tile([128, 128], F16, tag="eT")
                nc.vector.tensor_copy(out=sb_eT_t, in_=ps_tr)
                nc.tensor.matmul(
                    ps_out, lhsT=sb_eT_t, rhs=sb_vals16[:, vt, :],
                    start=(vt == 0), stop=(vt == VT - 1),
                    skip_group_check=True,
                )
            sb_out = work.tile([128, H], F32)
            nc.vector.tensor_scalar_mul(sb_out, ps_out, sb_rsum)
            nc.sync.dma_start(out=out_ext[:], in_=sb_out)

    nc.finalize()
    return nc


_NC_CACHE = {}


def _get_nc():
    if "nc" not in _NC_CACHE:
        _NC_CACHE["nc"] = build_nc()
    return _NC_CACHE["nc"]


def make_in_maps(queries, values, w1, w2, v):
    w1s = np.ascontiguousarray(w1, np.float32).reshape(HT, 128, U)
    w2s = np.ascontiguousarray(w2, np.float32).reshape(HT, 128, U)
    vs = np.ascontiguousarray(v, np.float32).reshape(U, 1)
    queries = np.asarray(queries, np.float32)
    values = np.asarray(values, np.float32)
    in_maps = []
    for c in range(8):
        b, qh = c // 2, c % 2
        q_shard = queries[b, qh * QL:(qh + 1) * QL, :]        # [QL, H]
        vb = values[b]                                        # [V, H]
        vbT = np.ascontiguousarray(vb.T)                      # [H, V]
        valsT = np.ascontiguousarray(
            vbT.reshape(HT, 128, NB, 512).transpose(2, 0, 1, 3))
        in_maps.append({
            "qT": np.ascontiguousarray(q_shard.T).reshape(HT, 128, QL),
            "valsT": valsT,
            "vals16": np.ascontiguousarray(vb.astype(np.float16)).reshape(VT, 128, H),
            "w1": w1s, "w2": w2s, "v": vs,
        })
    return in_maps


def gather_out(results):
    out = np.empty((B, Q, H), np.float32)
    for c in range(8):
        b, qh = c // 2, c % 2
        out[b, qh * QL:(qh + 1) * QL, :] = results[c]["out"]
    return out


def kernel(queries, values, w1, w2, v):
    from concourse.bass_utils import run_bass_kernel_spmd

    nc = _get_nc()
    in_maps = make_in_maps(queries, values, w1, w2, v)
    res = run_bass_kernel_spmd(nc, in_maps, list(range(8)))
    return gather_out(res.results)
